# revision 3
# baseline (speedup 1.0000x reference)
"""HalfEdgeConv Trainium2 kernel, v3.

out[e] = relu(W @ concat(x[next_idx[e]], has_twin[e] ? x[twin_idx[e]] : 0) + b)

Strategy (8 cores, data-parallel over half-edges):
  The baseline was Pool-SWDGE-bound: ~1490 indirect gather DMAs per core at
  ~1us fixed cost each (994ns SWDGE overhead per call, [128,1]-offset form
  is the only one the DGE executes faithfully).  v2 eliminates the next-side
  gathers entirely and halves the twin-side calls:

  - Edges are assigned to cores by next-row range: core c owns vertex rows
    [c*RPC, (c+1)*RPC) (RPC = 984 windows of 128 rows).  The slice of x a
    core needs for next-features is CONTIGUOUS, so it is streamed with a few
    big HWDGE DMAs (no per-row descriptors) as host-pre-transposed 65x128
    blocks (64 channels + a ones row).  A pre-pass matmul per window builds
    u = x @ W1.T + b, kept resident in SBUF (bf16, ~122KB/partition).
  - Each tile of 128 edges covers one 256-row segment of u.  Row selection
    is a one-hot S-matmul: S[p,e] = (r_e[e] == 128k+p) built on-chip by DVE
    is_equal against an uploaded iota column, with r_e uploaded
    pre-broadcast.  Two K=128 matmuls per tile accumulate u rows into PSUM.
  - Edges in a segment are split into a dead-twin tile and a live-twin tile;
    only live tiles issue the [128,1] twin gather (bf16 rows) ->
    ~492+~70 Pool calls instead of 1490 (the [128,1] gather call cadence
    of ~1.4us on the Pool engine is the remaining wall).  Gathered twin rows are PE
    transposed and accumulated into the same PSUM via a K=64 matmul.
  - Segment overflow edges (beyond 128 per lane) first spill into the same
    segment's live tile spare slots (free), then into overflow tiles sized
    at build time from the actual input (the program compiles after seeing
    the indices): dead-overflow tiles need only the next gather; live-
    overflow tiles gather next+twin; bias comes via a K=1 ones-matmul.
  - Everything is bf16 on the wire (x table, u, S, stores); PSUM accum f32.
    The kernel program is identical across cores; all data-dependent
    structure lives in per-core uploaded index/selector tensors.
"""
import os
import sys

sys.path.insert(0, "/opt/trn_rl_repo")

import numpy as np
from contextlib import ExitStack

import concourse.bass as bass
import concourse.tile as tile
from concourse import bacc, mybir, bass_utils

N = 1_000_000
C = 64
NCORES = 8
P = 128
WPC = 984                  # windows (128 rows) per core; divisible by UG
RPC = WPC * P              # 125952 rows per core
NSEG = WPC // 2            # 492 two-window segments per core
NTAB = NCORES * RPC + P    # full gather table rows (zeros beyond N)
ZROW = N                   # a guaranteed zero row for dead/pad gathers
QG = 8                     # tiles per PSUM output group
UG = 8                     # windows per u PSUM group
XCH = 32                   # windows per x2 stream chunk

f32 = mybir.dt.float32
bf16 = mybir.dt.bfloat16
i32 = mybir.dt.int32
i16 = mybir.dt.int16

_COMPILED = None
LAST_EXEC_NS = None


def _try_install_ntff_shim():
    """NTFF profiling hook (trace runs only); degrade silently if absent."""
    import types, ctypes, contextlib
    if "antenv.axon_hooks" in sys.modules:
        return
    try:
        import antenv
        mod = types.ModuleType("antenv.axon_hooks")
        mod._hook = None
        mod.set_axon_ntff_profile_hook = lambda h: setattr(mod, "_hook", h)
        mod.get_axon_ntff_profile_hook = lambda: mod._hook
        sys.modules["antenv.axon_hooks"] = mod
        antenv.axon_hooks = mod
        lib = ctypes.CDLL("/opt/axon/libaxon_pjrt.so")
        if not hasattr(lib, "axon_start_nrt_profile"):
            return
        lib.axon_start_nrt_profile.argtypes = [ctypes.POINTER(ctypes.c_int64), ctypes.c_size_t]
        lib.axon_start_nrt_profile.restype = ctypes.c_int64
        lib.axon_stop_nrt_profile.argtypes = [ctypes.c_char_p]
        lib.axon_stop_nrt_profile.restype = ctypes.c_int64

        @contextlib.contextmanager
        def _hook(output_dir, device_ids):
            import jax
            jax.devices()
            if device_ids:
                ids = (ctypes.c_int64 * len(device_ids))(*device_ids)
                rc = lib.axon_start_nrt_profile(ids, len(device_ids))
            else:
                rc = lib.axon_start_nrt_profile(None, 0)
            if rc != 0:
                raise RuntimeError(f"axon_start_nrt_profile rc={rc}")
            try:
                yield
            finally:
                lib.axon_stop_nrt_profile(str(output_dir).encode())

        mod.set_axon_ntff_profile_hook(_hook)
    except Exception:
        pass


def _build(novf_d, novf_l):
    novf = novf_d + novf_l
    TILES = NSEG * 2 + novf
    NLIVECOL = NSEG + novf_l
    nc = bacc.Bacc("TRN2", target_bir_lowering=False, debug=False)
    x2_d = nc.dram_tensor("x2", [WPC, C + 1, P], bf16, kind="ExternalInput").ap()
    xf_d = nc.dram_tensor("xf", [NTAB, C], bf16, kind="ExternalInput").ap()
    rb_d = nc.dram_tensor("rebc", [P, (NSEG * 2) * P], bf16, kind="ExternalInput").ap()
    io_d = nc.dram_tensor("io2", [P, 2], f32, kind="ExternalInput").ap()
    ti_d = nc.dram_tensor("ti", [P, NLIVECOL], i32, kind="ExternalInput").ap()
    nx_d = nc.dram_tensor("nx", [P, novf], i32, kind="ExternalInput").ap()
    w1t_d = nc.dram_tensor("w1t", [C + 1, C], bf16, kind="ExternalInput").ap()
    w2t_d = nc.dram_tensor("w2t", [C, C], bf16, kind="ExternalInput").ap()
    id_d = nc.dram_tensor("idn", [P, P], bf16, kind="ExternalInput").ap()
    br_d = nc.dram_tensor("brow", [1, C], bf16, kind="ExternalInput").ap()
    on_d = nc.dram_tensor("ones1", [1, P], bf16, kind="ExternalInput").ap()
    out_d = nc.dram_tensor("out", [P, TILES * C], bf16, kind="ExternalOutput").ap()

    with tile.TileContext(nc) as tc:
        with ExitStack() as ctx:
            const = ctx.enter_context(tc.tile_pool(name="const", bufs=1))
            upool = ctx.enter_context(tc.tile_pool(name="u", bufs=WPC // UG + 1))
            xtp = ctx.enter_context(tc.tile_pool(name="xt", bufs=2))
            rbp = ctx.enter_context(tc.tile_pool(name="rb", bufs=4))
            sp = ctx.enter_context(tc.tile_pool(name="s", bufs=6))
            twp = ctx.enter_context(tc.tile_pool(name="tw", bufs=96))
            twtp = ctx.enter_context(tc.tile_pool(name="twt", bufs=4))
            otp = ctx.enter_context(tc.tile_pool(name="ot", bufs=3))
            ups = ctx.enter_context(tc.tile_pool(name="ups", bufs=2, space="PSUM"))
            tps = ctx.enter_context(tc.tile_pool(name="tps", bufs=2, space="PSUM"))
            pop = ctx.enter_context(tc.tile_pool(name="po", bufs=2, space="PSUM"))

            w1t_sb = const.tile([C + 1, C], bf16)
            nc.sync.dma_start(w1t_sb[:], w1t_d[:])
            w2t_sb = const.tile([C, C], bf16)
            nc.sync.dma_start(w2t_sb[:], w2t_d[:])
            io_sb = const.tile([P, 2], f32)
            nc.sync.dma_start(io_sb[:], io_d[:])
            id_sb = const.tile([P, P], bf16)
            nc.sync.dma_start(id_sb[:], id_d[:])
            br_sb = const.tile([1, C], bf16)
            nc.sync.dma_start(br_sb[:], br_d[:])
            on_sb = const.tile([1, P], bf16)
            nc.sync.dma_start(on_sb[:], on_d[:])
            ti_sb = const.tile([P, NLIVECOL], i32)
            nc.sync.dma_start(ti_sb[:], ti_d[:])
            nx_sb = const.tile([P, novf], i32)
            nc.sync.dma_start(nx_sb[:], nx_d[:])

            # ---- pre-pass: u[w] = x_win[w] @ W1.T + b, resident in SBUF ----
            u_tiles = []
            ng = WPC // UG  # u groups; WPC divisible by UG
            assert WPC % UG == 0 and UG % 2 == 0
            for g in range(0, WPC, XCH):
                cw = min(XCH, WPC - g)
                xt = xtp.tile([C + 1, cw, P], bf16, tag="xt", name="xt")
                nc.sync.dma_start(
                    xt[:], x2_d[g:g + cw].rearrange("w p f -> p w f"))
                for g2 in range(g, g + cw, UG):
                    up = ups.tile([P, UG, C], f32, tag="ups", space="PSUM")
                    for j in range(UG):
                        nc.tensor.matmul(
                            out=up[:, j, :], lhsT=xt[:, g2 - g + j, :],
                            rhs=w1t_sb[:], start=True, stop=True)
                    ut = upool.tile([P, UG, C], bf16, tag="u", name="u")
                    nc.vector.tensor_copy(ut[:], up[:])
                    u_tiles.append(ut)

            # ---- main loop: QG tiles per PSUM group ----
            # tile order: (dead_0, live_0, dead_1, live_1, ..., ovf_0..)
            # regular groups have exactly QG//2 live tiles at odd positions
            for q0 in range(0, TILES, QG):
                po = pop.tile([P, QG, C], f32, tag="po", space="PSUM")
                nreg = min(max(NSEG * 2 - q0, 0), QG)
                if nreg > 0:
                    rb = rbp.tile([P, QG * P], bf16, tag="rb", name="rb")
                    nc.sync.dma_start(
                        rb[:, 0:nreg * P],
                        rb_d[:, q0 * P:(q0 + nreg) * P])

                # phase A: twin gathers + transposes for the group's regular
                # live tiles (odd qi), one batched PSUM eviction
                live_qis = [qi for qi in range(QG)
                            if q0 + qi < NSEG * 2 and (q0 + qi) % 2 == 1]
                twtg = None
                if live_qis:
                    tpg = tps.tile([P, len(live_qis), P], bf16, tag="tp",
                                   space="PSUM")
                    for si, qi in enumerate(live_qis):
                        seg = (q0 + qi) // 2
                        tw = twp.tile([P, C], bf16, tag="tw", name="tw")
                        nc.gpsimd.indirect_dma_start(
                            out=tw[:], out_offset=None, in_=xf_d[:],
                            in_offset=bass.IndirectOffsetOnAxis(
                                ap=ti_sb[:, seg:seg + 1], axis=0))
                        nc.tensor.transpose(
                            out=tpg[:C, si, :], in_=tw[:], identity=id_sb[:])
                    twtg = twtp.tile([C, len(live_qis), P], bf16,
                                     tag="twt", name="twt")
                    nc.vector.tensor_copy(twtg[:], tpg[:C, :, :])

                # phase B: S builds + matmul accumulation per tile
                for qi in range(QG):
                    t = q0 + qi
                    if t < NSEG * 2:
                        seg, live = divmod(t, 2)
                        s_all = sp.tile([P, 2, P], bf16, tag="s", name="s")
                        for k in range(2):
                            nc.vector.tensor_scalar(
                                out=s_all[:, k, :],
                                in0=rb[:, qi * P:(qi + 1) * P],
                                scalar1=io_sb[:, k:k + 1], scalar2=None,
                                op0=mybir.AluOpType.is_equal)
                        w0 = 2 * seg
                        ug, us = divmod(w0, UG)
                        for k in range(2):
                            nc.tensor.matmul(
                                out=po[:, qi, :], lhsT=s_all[:, k, :],
                                rhs=u_tiles[ug][:, us + k, :],
                                start=(k == 0), stop=(k == 1 and not live))
                        if live:
                            nc.tensor.matmul(
                                out=po[:, qi, :],
                                lhsT=twtg[:, live_qis.index(qi), :],
                                rhs=w2t_sb[:], start=False, stop=True)
                    else:
                        # overflow tile: next gather always, twin only if live
                        o = t - NSEG * 2
                        has_tw = o >= novf_d
                        nslot = 2 if has_tw else 1
                        nf = twp.tile([P, C], bf16, tag="tw", name="nf")
                        nc.gpsimd.indirect_dma_start(
                            out=nf[:], out_offset=None, in_=xf_d[:],
                            in_offset=bass.IndirectOffsetOnAxis(
                                ap=nx_sb[:, o:o + 1], axis=0))
                        tp = tps.tile([P, 2, P], bf16, tag="tp2", space="PSUM")
                        nc.tensor.transpose(
                            out=tp[:C, 0, :], in_=nf[:], identity=id_sb[:])
                        if has_tw:
                            tw = twp.tile([P, C], bf16, tag="tw", name="tw")
                            nc.gpsimd.indirect_dma_start(
                                out=tw[:], out_offset=None, in_=xf_d[:],
                                in_offset=bass.IndirectOffsetOnAxis(
                                    ap=ti_sb[:, NSEG + o - novf_d:
                                             NSEG + o - novf_d + 1], axis=0))
                            nc.tensor.transpose(
                                out=tp[:C, 1, :], in_=tw[:], identity=id_sb[:])
                        twt = twtp.tile([C, 2, P], bf16, tag="twt2", name="twt2")
                        nc.vector.tensor_copy(
                            twt[:, 0:nslot, :], tp[:C, 0:nslot, :])
                        nc.tensor.matmul(
                            out=po[:, qi, :], lhsT=twt[:, 0, :],
                            rhs=w1t_sb[:C, :], start=True, stop=False)
                        if has_tw:
                            nc.tensor.matmul(
                                out=po[:, qi, :], lhsT=twt[:, 1, :],
                                rhs=w2t_sb[:], start=False, stop=False)
                        nc.tensor.matmul(
                            out=po[:, qi, :], lhsT=on_sb[:],
                            rhs=br_sb[:], start=False, stop=True)

                ot = otp.tile([P, QG * C], bf16, tag="ot")
                nc.scalar.activation(
                    ot[:], po[:].rearrange("p t c -> p (t c)"),
                    mybir.ActivationFunctionType.Relu)
                nc.sync.dma_start(out_d[:, q0 * C:(q0 + QG) * C], ot[:])

    nc.compile()
    return nc


def _get_compiled(novf_d, novf_l):
    global _COMPILED
    if _COMPILED is None:
        _COMPILED = _build(novf_d, novf_l)
    return _COMPILED


def kernel(x, next_idx, twin_idx, has_twin, W, b):
    global LAST_EXEC_NS
    import ml_dtypes
    bf = ml_dtypes.bfloat16

    x = np.asarray(x, dtype=np.float32)
    next_idx = np.asarray(next_idx, dtype=np.int64)
    twin_idx = np.asarray(twin_idx, dtype=np.int64)
    has_twin = np.asarray(has_twin).astype(bool)
    W = np.asarray(W, dtype=np.float32)
    b = np.asarray(b, dtype=np.float32)

    trace = bool(os.environ.get("BASS_TRACE"))
    if trace:
        _try_install_ntff_shim()

    # ---- host packing ----
    core = next_idx // RPC
    local = next_idx - core * RPC
    seg = local >> 8                     # 256-row segment within core
    lane = has_twin.astype(np.int64)     # 1 = live twin
    bucket = (core * NSEG + seg) * 2 + lane
    order = np.argsort(bucket, kind="stable")
    bc_sorted = bucket[order]
    counts = np.bincount(bucket, minlength=NCORES * NSEG * 2 + 1)
    starts = np.zeros_like(counts)
    np.cumsum(counts[:-1], out=starts[1:])
    rank = np.arange(N, dtype=np.int64) - starts[bc_sorted]

    in_tile = rank < P
    e_in = order[in_tile]                # edges placed in regular tiles
    t_in = (bc_sorted[in_tile] % (NSEG * 2))      # tile within core
    c_in = bc_sorted[in_tile] // (NSEG * 2)       # core
    s_in = rank[in_tile]                 # slot

    # dead-lane excess routed into the same segment's live tile spare slots
    # (those edges keep twin = ZROW; the live tile gathers anyway)
    ov_mask = ~in_tile
    dead_ov = ov_mask & (bc_sorted % 2 == 0)
    live_cnt_of = counts[bc_sorted + 1]  # live-lane count of same segment
    spare = np.maximum(P - np.minimum(live_cnt_of, P), 0)
    jd = rank - P
    routed = dead_ov & (jd < spare)
    e_rt = order[routed]
    b_rt = bc_sorted[routed]
    t_rt = (b_rt % (NSEG * 2)) + 1       # the live tile of the segment
    c_rt = b_rt // (NSEG * 2)
    s_rt = np.minimum(live_cnt_of[routed], P) + jd[routed]
    e_in = np.concatenate([e_in, e_rt])
    t_in = np.concatenate([t_in, t_rt])
    c_in = np.concatenate([c_in, c_rt])
    s_in = np.concatenate([s_in, s_rt])

    ov_final = ov_mask & ~routed
    e_ov0 = order[ov_final]              # true overflow edges
    c_ov0 = bc_sorted[ov_final] // (NSEG * 2)
    l_ov0 = (bc_sorted[ov_final] % 2).astype(np.int64)  # 1 = live twin
    # dead-ovf tiles hold only dead edges (one gather); the rest go to
    # live-ovf tiles (two gathers; leftover dead edges get ZROW twins)
    dead_counts = np.bincount(c_ov0[l_ov0 == 0], minlength=NCORES)
    ov_counts = np.bincount(c_ov0, minlength=NCORES)
    novf_d = int(dead_counts.min()) // P
    rest_max = int((ov_counts - novf_d * P).max())
    novf_l = -(-(rest_max + 192) // P)
    while (NSEG * 2 + novf_d + novf_l) % QG:
        novf_l += 1
    novf = novf_d + novf_l
    TILES = NSEG * 2 + novf
    NLIVECOL = NSEG + novf_l
    # order overflow edges per core: dead first, then live
    ov_order = np.lexsort((l_ov0, c_ov0))
    e_ov = e_ov0[ov_order]
    c_ov = c_ov0[ov_order]
    ov_starts = np.zeros_like(ov_counts)
    np.cumsum(ov_counts[:-1], out=ov_starts[1:])
    r_ov = np.arange(e_ov.size, dtype=np.int64) - ov_starts[c_ov]
    t_ov = NSEG * 2 + r_ov // P
    s_ov = r_ov % P

    # full bf16 gather table (zeros beyond N; ZROW guaranteed zero)
    xf = np.zeros((NTAB, C), dtype=bf)
    xf[:N] = x.astype(bf)

    # per-core pre-transposed x blocks with ones row
    xpad = np.zeros((NCORES * RPC, C), np.float32)
    xpad[:N] = x
    x2_all = np.empty((NCORES, WPC, C + 1, P), dtype=bf)
    blocks = xpad.reshape(NCORES, WPC, P, C)
    x2_all[:, :, :C, :] = blocks.transpose(0, 1, 3, 2).astype(bf)
    x2_all[:, :, C, :] = np.float32(1.0)

    w1t = np.concatenate([W[:, :C].T, b[None, :]], axis=0).astype(bf)
    w2t = np.ascontiguousarray(W[:, C:].T).astype(bf)
    io2 = (np.arange(P, dtype=np.float32)[:, None]
           + 128.0 * np.arange(2, dtype=np.float32)[None, :]).astype(np.float32)
    idn = np.eye(P, dtype=np.float32).astype(bf)
    brow = b[None, :].astype(bf)
    ones1 = np.ones((1, P), dtype=bf)

    re_all = np.full((NCORES, NSEG * 2 * P), 300.0, bf)
    re_all[c_in, t_in * P + s_in] = (local[e_in] - (t_in // 2) * 256).astype(bf)
    ti_all = np.full((NCORES, NLIVECOL, P), ZROW, np.int64)
    live_mask = lane[e_in] == 1
    ti_all[c_in[live_mask], t_in[live_mask] // 2, s_in[live_mask]] = \
        twin_idx[e_in[live_mask]]
    ov_live = has_twin[e_ov]
    assert not np.any(ov_live & (t_ov < NSEG * 2 + novf_d)), \
        "live edge landed in a dead-ovf tile"
    ti_all[c_ov[ov_live], NSEG + (t_ov[ov_live] - NSEG * 2 - novf_d),
           s_ov[ov_live]] = twin_idx[e_ov[ov_live]]
    nx_all = np.full((NCORES, novf, P), ZROW, np.int64)
    nx_all[c_ov, t_ov - NSEG * 2, s_ov] = next_idx[e_ov]

    # output slot map
    emap = np.full((NCORES, TILES, P), -1, np.int64)
    emap[c_in, t_in, s_in] = e_in
    emap[c_ov, t_ov, s_ov] = e_ov

    in_maps = []
    for c in range(NCORES):
        in_maps.append({
            "x2": x2_all[c],
            "xf": xf,
            "rebc": np.broadcast_to(re_all[c][None, :], (P, NSEG * 2 * P)).copy(),
            "io2": io2,
            "ti": np.ascontiguousarray(ti_all[c].T).astype(np.int32),
            "nx": np.ascontiguousarray(nx_all[c].T).astype(np.int32),
            "w1t": w1t, "w2t": w2t, "idn": idn, "brow": brow, "ones1": ones1,
        })

    nc = _get_compiled(novf_d, novf_l)
    res = bass_utils.run_bass_kernel_spmd(
        nc, in_maps, core_ids=list(range(NCORES)), trace=trace)
    LAST_EXEC_NS = res.exec_time_ns

    out = np.empty((N, C), np.float32)
    for c in range(NCORES):
        o = np.asarray(res.results[c]["out"]).astype(np.float32)
        o = o.reshape(P, TILES, C).transpose(1, 0, 2).reshape(TILES * P, C)
        m = emap[c].reshape(-1)
        sel = m >= 0
        out[m[sel]] = o[sel]
    return out
